# revision 6
# baseline (speedup 1.0000x reference)
"""CPRNN Trainium2 kernel (8-core SPMD).

Strategy:
  - The CP recurrence (sequential over S=256) is replicated on every core
    in fully-transposed orientation: h lives as hT [H, B] fp16 tiles, so
    mm1 (h@a) uses stationary `a` tiles, mm2 uses stationary `c.T` tiles,
    and tanh emits hT directly -- no per-step transposes.
  - The dominant decoder matmul [S*B, H] @ [H, V] is tensor-parallel over
    the vocab dim: every core runs the same program but receives its own
    dec_w.T shard (V/8 = 4000 columns) and writes its own logits shard.
  - bp = (emb[inp]) @ b is computed on-device per core: indirect-DMA row
    gather of fp16 emb -> PE transpose -> b-stationary matmul -> bpT fp16,
    pipelined 2 groups ahead of the recurrence.
  - All matmuls fp16 (1 cycle/row + fast weight load), fp32 PSUM
    accumulation; logits bias-add happens in fp32.

Self-contained: hardcodes all shapes; host code only reshapes/transposes.
"""
import sys
sys.path.insert(0, "/opt/trn_rl_repo")
import numpy as np

S, B, D, H, R, V = 256, 32, 1024, 1024, 256, 32000
NCORES = 8
VSH = V // NCORES          # 4000 vocab columns per core
SB = S * B                 # 8192 token rows
NG = S // 4                # 64 groups of 4 steps = 128 token rows
KD = D // 128              # 8 contraction tiles over D/H
KR = R // 128              # 2 contraction tiles over R
NT = VSH // 8              # 500 decoder free-dim columns per n-tile
LEAD = 2                   # bp chunks emitted this many groups ahead
H0VAL = 0.0                # initial hidden state fill (nonzero only for validation)
NBODY = 1                  # repeat whole body N times (timing: marginal = HW time)

_CACHED = {}


def _build_nc():
    import concourse.bass as bass
    import concourse.bacc as bacc
    import concourse.mybir as mybir
    import concourse.tile as tile

    fp16, fp32, i32 = mybir.dt.float16, mybir.dt.float32, mybir.dt.int32
    Tanh = mybir.ActivationFunctionType.Tanh

    nc = bacc.Bacc(None, target_bir_lowering=False)
    emb16 = nc.dram_tensor("emb16", [V, D], fp16, kind="ExternalInput")
    idx32 = nc.dram_tensor("idx32", [128, NG], i32, kind="ExternalInput")
    a16 = nc.dram_tensor("a16", [H, R], fp16, kind="ExternalInput")
    b16 = nc.dram_tensor("b16", [D, R], fp16, kind="ExternalInput")
    cT16 = nc.dram_tensor("cT16", [R, H], fp16, kind="ExternalInput")
    ident = nc.dram_tensor("ident", [128, 128], fp16, kind="ExternalInput")
    dwT = nc.dram_tensor("dwT", [H, VSH], fp16, kind="ExternalInput")
    brep = nc.dram_tensor("brep", [128, VSH], fp32, kind="ExternalInput")

    logits_sh = nc.dram_tensor("logits_sh", [SB, VSH], fp32, kind="ExternalOutput")
    hlastT = nc.dram_tensor("hlastT", [H, B], fp32, kind="ExternalOutput")

    with tile.TileContext(nc) as tc:
        with (
            tc.tile_pool(name="cst", bufs=1) as cst,
            tc.tile_pool(name="xg", bufs=3) as xgp,
            tc.tile_pool(name="xT", bufs=2) as xTp,
            tc.tile_pool(name="ghat", bufs=2) as ghp,
            tc.tile_pool(name="hT", bufs=3) as hTp,
            tc.tile_pool(name="lg", bufs=3) as lgp,
            tc.tile_pool(name="ptr", bufs=2, space="PSUM") as ptr,   # 2 banks
            tc.tile_pool(name="pbp", bufs=1, space="PSUM") as pbp,   # 1 bank
            tc.tile_pool(name="pg", bufs=1, space="PSUM") as pgp,    # 1 bank
            tc.tile_pool(name="pu", bufs=1, space="PSUM") as pup,    # 1 bank
            tc.tile_pool(name="pd", bufs=3, space="PSUM") as pdp,    # 3 banks
        ):
            # ---- constants into SBUF ----
            idx_t = cst.tile([128, NG], i32)
            nc.sync.dma_start(idx_t[:], idx32[:])
            id_t = cst.tile([128, 128], fp16)
            nc.sync.dma_start(id_t[:], ident[:])
            a_t = cst.tile([128, KD, R], fp16)
            nc.sync.dma_start(a_t[:], a16[:].rearrange("(k p) r -> p k r", p=128))
            b_t = cst.tile([128, KD, R], fp16)
            nc.sync.dma_start(b_t[:], b16[:].rearrange("(k p) r -> p k r", p=128))
            cT_t = cst.tile([128, KR, H], fp16)
            nc.sync.dma_start(cT_t[:], cT16[:].rearrange("(k p) h -> p k h", p=128))
            dw_t = cst.tile([128, KD, VSH], fp16)
            nc.sync.dma_start(dw_t[:], dwT[:].rearrange("(k p) v -> p k v", p=128))
            bias_t = cst.tile([128, VSH], fp32)
            nc.sync.dma_start(bias_t[:], brep[:])
            bpT = cst.tile([128, KR, SB], fp16)   # bp transposed: [R, tokens]
            h0_t = cst.tile([128, KD, B], fp16)
            nc.gpsimd.memset(h0_t[:], H0VAL)
            hlast_sb = cst.tile([128, KD, B], fp32)

            def bp_chunk(g):
                # gather 128 token rows of emb, transpose, project to bpT cols
                xg = xgp.tile([128, D], fp16, tag="xg")
                nc.gpsimd.indirect_dma_start(
                    out=xg[:], out_offset=None, in_=emb16[:],
                    in_offset=bass.IndirectOffsetOnAxis(ap=idx_t[:, g:g + 1], axis=0),
                )
                xT = xTp.tile([128, KD, 128], fp16, tag="xT")
                for k in range(KD):
                    tp = ptr.tile([128, 128], fp16, tag="tp")
                    nc.tensor.transpose(tp[:], xg[:, k * 128:(k + 1) * 128], id_t[:])
                    nc.any.tensor_copy(xT[:, k, :], tp[:])
                for m in range(KR):
                    acc = pbp.tile([128, 128], fp32, tag="bpacc")
                    for k in range(KD):
                        nc.tensor.matmul(acc[:], b_t[:, k, m * 128:(m + 1) * 128],
                                         xT[:, k, :], start=(k == 0), stop=(k == KD - 1))
                    nc.vector.tensor_copy(bpT[:, m, g * 128:(g + 1) * 128], acc[:])

            def body():
                for g in range(LEAD):
                    bp_chunk(g)
                hT_tiles = [None, None]  # [prev group tile, cur group tile]
                for g in range(NG):
                    if g + LEAD < NG:
                        bp_chunk(g + LEAD)
                    hTg = hTp.tile([128, KD, 128], fp16, tag="hT")
                    hT_tiles = [hT_tiles[1], hTg]
                    for s in range(4):
                        t = 4 * g + s
                        # previous hidden state (output of step t-1)
                        if t == 0:
                            hprev, ps = h0_t, 0
                        elif s == 0:
                            hprev, ps = hT_tiles[0], 3
                        else:
                            hprev, ps = hTg, s - 1
                        # mm1: gT[R,B] = a.T @ hT  (stationary a tiles)
                        psg = pgp.tile([128, KR * B], fp32, tag="psg")
                        for m in range(KR):
                            for k in range(KD):
                                nc.tensor.matmul(
                                    psg[:, m * B:(m + 1) * B],
                                    a_t[:, k, m * 128:(m + 1) * 128],
                                    hprev[:, k, ps * B:(ps + 1) * B],
                                    start=(k == 0), stop=(k == KD - 1))
                        # ghat = gT * bpT[:, :, t*B:(t+1)*B]  (cast to fp16)
                        gh = ghp.tile([128, KR * B], fp16, tag="gh")
                        nc.vector.tensor_mul(
                            gh[:].rearrange("p (m b) -> p m b", m=KR),
                            psg[:].rearrange("p (m b) -> p m b", m=KR),
                            bpT[:, :, t * B:(t + 1) * B])
                        # mm2: uT[H,B] = c @ ghat  (stationary cT tiles)
                        psu = pup.tile([128, KD * B], fp32, tag="psu")
                        for hh in range(KD):
                            for k in range(KR):
                                nc.tensor.matmul(
                                    psu[:, hh * B:(hh + 1) * B],
                                    cT_t[:, k, hh * 128:(hh + 1) * 128],
                                    gh[:, k * B:(k + 1) * B],
                                    start=(k == 0), stop=(k == KR - 1))
                        # tanh -> hT fp16 (decoder-ready layout)
                        for hh in range(KD):
                            nc.scalar.activation(hTg[:, hh, s * B:(s + 1) * B],
                                                 psu[:, hh * B:(hh + 1) * B], Tanh)
                        if t == S - 1:
                            for hh in range(KD):
                                nc.scalar.activation(hlast_sb[:, hh, :],
                                                     psu[:, hh * B:(hh + 1) * B], Tanh)
                    # decoder for this group's 128 token rows
                    for n in range(VSH // NT):
                        accd = pdp.tile([128, NT], fp32, tag="accd")
                        for k in range(KD):
                            nc.tensor.matmul(accd[:], hTg[:, k, :],
                                             dw_t[:, k, n * NT:(n + 1) * NT],
                                             start=(k == 0), stop=(k == KD - 1))
                        lg = lgp.tile([128, NT], fp32, tag="lg")
                        nc.vector.tensor_add(lg[:], accd[:],
                                             bias_t[:, n * NT:(n + 1) * NT])
                        nc.sync.dma_start(
                            logits_sh[g * 128:(g + 1) * 128, n * NT:(n + 1) * NT],
                            lg[:])
                for hh in range(KD):
                    nc.sync.dma_start(hlastT[hh * 128:(hh + 1) * 128, :],
                                      hlast_sb[:, hh, :])

            for _rep in range(NBODY):
                body()
    nc.finalize()
    return nc


def _get_nc():
    if "nc" not in _CACHED:
        _CACHED["nc"] = _build_nc()
    return _CACHED["nc"]


def _prep_in_maps(inp, emb, a, b, c, dec_w, dec_b):
    f16 = np.float16
    emb16 = np.ascontiguousarray(emb, dtype=f16)
    a16 = np.ascontiguousarray(a, dtype=f16)
    b16 = np.ascontiguousarray(b, dtype=f16)
    cT16 = np.ascontiguousarray(np.asarray(c, dtype=np.float32).T, dtype=f16)
    ident = np.eye(128, dtype=f16)
    # token (g*128 + p) at idx32[p, g]
    idx32 = np.ascontiguousarray(
        np.asarray(inp, dtype=np.int64).reshape(SB).reshape(NG, 128).T
    ).astype(np.int32)
    dwT = np.asarray(dec_w, dtype=np.float32).T  # [H, V]
    dec_b = np.asarray(dec_b, dtype=np.float32)
    common = dict(emb16=emb16, idx32=idx32, a16=a16, b16=b16, cT16=cT16, ident=ident)
    in_maps = []
    for core in range(NCORES):
        sl = slice(core * VSH, (core + 1) * VSH)
        m = dict(common)
        m["dwT"] = np.ascontiguousarray(dwT[:, sl], dtype=f16)
        m["brep"] = np.ascontiguousarray(
            np.broadcast_to(dec_b[sl][None, :], (128, VSH)), dtype=np.float32)
        in_maps.append(m)
    return in_maps


def run_on_hw(in_maps, **kwargs):
    from concourse.bass_utils import run_bass_kernel_spmd
    nc = _get_nc()
    return run_bass_kernel_spmd(nc, in_maps, list(range(NCORES)), **kwargs)


def kernel(inp, emb, a, b, c, dec_w, dec_b):
    in_maps = _prep_in_maps(inp, emb, a, b, c, dec_w, dec_b)
    res = run_on_hw(in_maps).results
    logits = np.concatenate([res[i]["logits_sh"] for i in range(NCORES)], axis=1)
    logits = np.ascontiguousarray(logits.reshape(S, B, V), dtype=np.float32)
    h_last = np.ascontiguousarray(res[0]["hlastT"].T, dtype=np.float32)
    return logits, h_last


# revision 9
# speedup vs baseline: 1.2488x; 1.2488x over previous
"""CPRNN Trainium2 kernel (8-core SPMD).

Strategy:
  - The CP recurrence (sequential over S=256) is replicated on every core
    in fully-transposed orientation: h lives as hT [H, B] fp16 tiles, so
    mm1 (h@a) uses stationary `a` tiles, mm2 uses stationary `c.T` tiles,
    and tanh emits hT directly -- no per-step transposes.
  - The dominant decoder matmul [S*B, H] @ [H, V] is tensor-parallel over
    the vocab dim: every core runs the same program but receives its own
    dec_w.T shard (V/8 = 4000 columns) and writes its own logits shard.
  - bp = (emb[inp]) @ b is computed on-device per core: indirect-DMA row
    gather of fp16 emb -> PE transpose -> b-stationary matmul -> bpT fp16,
    pipelined 2 groups ahead of the recurrence.
  - All matmuls fp16 (1 cycle/row + fast weight load), fp32 PSUM
    accumulation; logits bias-add happens in fp32.

Self-contained: hardcodes all shapes; host code only reshapes/transposes.
"""
import sys
sys.path.insert(0, "/opt/trn_rl_repo")
import numpy as np

S, B, D, H, R, V = 256, 32, 1024, 1024, 256, 32000
NCORES = 8
VSH = V // NCORES          # 4000 vocab columns per core
SB = S * B                 # 8192 token rows
NG = S // 4                # 64 groups of 4 steps = 128 token rows
KD = D // 128              # 8 contraction tiles over D/H
KR = R // 128              # 2 contraction tiles over R
NT = VSH // 8              # 500 decoder free-dim columns per n-tile
LEAD = 2                   # bp chunks emitted this many groups ahead
H0VAL = 0.0                # initial hidden state fill (nonzero only for validation)
NBODY = 1                  # repeat whole body N times (timing: marginal = HW time)
SKIP_DECODER = False       # timing variant: recurrence + bp only
SKIP_REC = False           # timing variant: decoder only (hT memset)

_CACHED = {}


def _build_nc():
    import concourse.bass as bass
    import concourse.bacc as bacc
    import concourse.mybir as mybir
    import concourse.tile as tile

    fp16, fp32, i32 = mybir.dt.float16, mybir.dt.float32, mybir.dt.int32
    Tanh = mybir.ActivationFunctionType.Tanh

    nc = bacc.Bacc(None, target_bir_lowering=False)
    emb16 = nc.dram_tensor("emb16", [V, D], fp16, kind="ExternalInput")
    idx32 = nc.dram_tensor("idx32", [128, NG], i32, kind="ExternalInput")
    a16 = nc.dram_tensor("a16", [H, R], fp16, kind="ExternalInput")
    b16 = nc.dram_tensor("b16", [D, R], fp16, kind="ExternalInput")
    cT16 = nc.dram_tensor("cT16", [R, H], fp16, kind="ExternalInput")
    ident = nc.dram_tensor("ident", [128, 128], fp16, kind="ExternalInput")
    dwT = nc.dram_tensor("dwT", [H, VSH], fp16, kind="ExternalInput")
    brep = nc.dram_tensor("brep", [128, VSH], fp32, kind="ExternalInput")

    logits_sh = nc.dram_tensor("logits_sh", [SB, VSH], fp32, kind="ExternalOutput")
    hlastT = nc.dram_tensor("hlastT", [H, B], fp32, kind="ExternalOutput")

    with tile.TileContext(nc) as tc:
        with (
            tc.tile_pool(name="cst", bufs=1) as cst,
            tc.tile_pool(name="xg", bufs=3) as xgp,
            tc.tile_pool(name="xT", bufs=2) as xTp,
            tc.tile_pool(name="ghat", bufs=2) as ghp,
            tc.tile_pool(name="hT", bufs=3) as hTp,
            tc.tile_pool(name="lg", bufs=3) as lgp,
            tc.tile_pool(name="ptr", bufs=2, space="PSUM") as ptr,   # 2 banks
            tc.tile_pool(name="pbp", bufs=1, space="PSUM") as pbp,   # 1 bank
            tc.tile_pool(name="pg", bufs=1, space="PSUM") as pgp,    # 1 bank
            tc.tile_pool(name="pu", bufs=1, space="PSUM") as pup,    # 1 bank
            tc.tile_pool(name="pd", bufs=3, space="PSUM") as pdp,    # 3 banks
        ):
            # ---- constants into SBUF ----
            idx_t = cst.tile([128, NG], i32)
            nc.sync.dma_start(idx_t[:], idx32[:])
            id_t = cst.tile([128, 128], fp16)
            nc.sync.dma_start(id_t[:], ident[:])
            a_t = cst.tile([128, KD, R], fp16)
            nc.sync.dma_start(a_t[:], a16[:].rearrange("(k p) r -> p k r", p=128))
            b_t = cst.tile([128, KD, R], fp16)
            nc.sync.dma_start(b_t[:], b16[:].rearrange("(k p) r -> p k r", p=128))
            cT_t = cst.tile([128, KR, H], fp16)
            nc.sync.dma_start(cT_t[:], cT16[:].rearrange("(k p) h -> p k h", p=128))
            dw_t = cst.tile([128, KD, VSH], fp16)
            nc.sync.dma_start(dw_t[:], dwT[:].rearrange("(k p) v -> p k v", p=128))
            bias_t = cst.tile([128, VSH], fp32)
            nc.sync.dma_start(bias_t[:], brep[:])
            bpT = cst.tile([128, KR, SB], fp16)   # bp transposed: [R, tokens]
            h0_t = cst.tile([128, KD, B], fp16)
            nc.gpsimd.memset(h0_t[:], H0VAL)
            hlast_sb = cst.tile([128, KD, B], fp32)

            def bp_chunk(g):
                # gather 128 token rows of emb, transpose, project to bpT cols
                xg = xgp.tile([128, D], fp16, tag="xg")
                nc.gpsimd.indirect_dma_start(
                    out=xg[:], out_offset=None, in_=emb16[:],
                    in_offset=bass.IndirectOffsetOnAxis(ap=idx_t[:, g:g + 1], axis=0),
                )
                xT = xTp.tile([128, KD, 128], fp16, tag="xT")
                for k in range(KD):
                    tp = ptr.tile([128, 128], fp16, tag="tp")
                    nc.tensor.transpose(tp[:], xg[:, k * 128:(k + 1) * 128], id_t[:])
                    nc.any.tensor_copy(xT[:, k, :], tp[:])
                for m in range(KR):
                    acc = pbp.tile([128, 128], fp32, tag="bpacc")
                    for k in range(KD):
                        nc.tensor.matmul(acc[:], b_t[:, k, m * 128:(m + 1) * 128],
                                         xT[:, k, :], start=(k == 0), stop=(k == KD - 1))
                    nc.vector.tensor_copy(bpT[:, m, g * 128:(g + 1) * 128], acc[:])

            def body():
                if not SKIP_REC:
                    for g in range(LEAD):
                        bp_chunk(g)
                hT_tiles = [None, None]  # [prev group tile, cur group tile]
                for g in range(NG):
                    if g + LEAD < NG and not SKIP_REC:
                        bp_chunk(g + LEAD)
                    hTg = hTp.tile([128, KD, 128], fp16, tag="hT")
                    hT_tiles = [hT_tiles[1], hTg]
                    if SKIP_REC:
                        nc.gpsimd.memset(hTg[:], 0.0)
                    for s in [] if SKIP_REC else range(4):
                        t = 4 * g + s
                        # previous hidden state (output of step t-1)
                        if t == 0:
                            hprev, ps = h0_t, 0
                        elif s == 0:
                            hprev, ps = hT_tiles[0], 3
                        else:
                            hprev, ps = hTg, s - 1
                        # mm1: gT[R,B] = a.T @ hT  (stationary a tiles)
                        psg = pgp.tile([128, KR * B], fp32, tag="psg")
                        for m in range(KR):
                            for k in range(KD):
                                nc.tensor.matmul(
                                    psg[:, m * B:(m + 1) * B],
                                    a_t[:, k, m * 128:(m + 1) * 128],
                                    hprev[:, k, ps * B:(ps + 1) * B],
                                    start=(k == 0), stop=(k == KD - 1))
                        # ghat = gT * bpT[:, :, t*B:(t+1)*B]  (cast to fp16)
                        gh = ghp.tile([128, KR * B], fp16, tag="gh")
                        nc.vector.tensor_mul(
                            gh[:].rearrange("p (m b) -> p m b", m=KR),
                            psg[:].rearrange("p (m b) -> p m b", m=KR),
                            bpT[:, :, t * B:(t + 1) * B])
                        # mm2: uT[H,B] = c @ ghat  (stationary cT tiles)
                        psu = pup.tile([128, KD * B], fp32, tag="psu")
                        for hh in range(KD):
                            for k in range(KR):
                                nc.tensor.matmul(
                                    psu[:, hh * B:(hh + 1) * B],
                                    cT_t[:, k, hh * 128:(hh + 1) * 128],
                                    gh[:, k * B:(k + 1) * B],
                                    start=(k == 0), stop=(k == KR - 1))
                        # tanh -> hT fp16 (decoder-ready layout)
                        for hh in range(KD):
                            nc.scalar.activation(hTg[:, hh, s * B:(s + 1) * B],
                                                 psu[:, hh * B:(hh + 1) * B], Tanh)
                        if t == S - 1:
                            for hh in range(KD):
                                nc.scalar.activation(hlast_sb[:, hh, :],
                                                     psu[:, hh * B:(hh + 1) * B], Tanh)
                    # decoder for this group's 128 token rows
                    for n in [] if SKIP_DECODER else range(VSH // NT):
                        accd = pdp.tile([128, NT], fp32, tag="accd")
                        for k in range(KD):
                            nc.tensor.matmul(accd[:], hTg[:, k, :],
                                             dw_t[:, k, n * NT:(n + 1) * NT],
                                             start=(k == 0), stop=(k == KD - 1))
                        lg = lgp.tile([128, NT], fp32, tag="lg")
                        nc.vector.tensor_add(lg[:], accd[:],
                                             bias_t[:, n * NT:(n + 1) * NT])
                        nc.sync.dma_start(
                            logits_sh[g * 128:(g + 1) * 128, n * NT:(n + 1) * NT],
                            lg[:])
                for hh in range(KD):
                    nc.sync.dma_start(hlastT[hh * 128:(hh + 1) * 128, :],
                                      hlast_sb[:, hh, :])

            for _rep in range(NBODY):
                body()
    nc.finalize()
    return nc


def _get_nc():
    if "nc" not in _CACHED:
        _CACHED["nc"] = _build_nc()
    return _CACHED["nc"]


def _prep_in_maps(inp, emb, a, b, c, dec_w, dec_b):
    f16 = np.float16
    emb16 = np.ascontiguousarray(emb, dtype=f16)
    a16 = np.ascontiguousarray(a, dtype=f16)
    b16 = np.ascontiguousarray(b, dtype=f16)
    cT16 = np.ascontiguousarray(np.asarray(c, dtype=np.float32).T, dtype=f16)
    ident = np.eye(128, dtype=f16)
    # token (g*128 + p) at idx32[p, g]
    idx32 = np.ascontiguousarray(
        np.asarray(inp, dtype=np.int64).reshape(SB).reshape(NG, 128).T
    ).astype(np.int32)
    dwT = np.asarray(dec_w, dtype=np.float32).T  # [H, V]
    dec_b = np.asarray(dec_b, dtype=np.float32)
    common = dict(emb16=emb16, idx32=idx32, a16=a16, b16=b16, cT16=cT16, ident=ident)
    in_maps = []
    for core in range(NCORES):
        sl = slice(core * VSH, (core + 1) * VSH)
        m = dict(common)
        m["dwT"] = np.ascontiguousarray(dwT[:, sl], dtype=f16)
        m["brep"] = np.ascontiguousarray(
            np.broadcast_to(dec_b[sl][None, :], (128, VSH)), dtype=np.float32)
        in_maps.append(m)
    return in_maps


def run_on_hw(in_maps, **kwargs):
    from concourse.bass_utils import run_bass_kernel_spmd
    nc = _get_nc()
    return run_bass_kernel_spmd(nc, in_maps, list(range(NCORES)), **kwargs)


def kernel(inp, emb, a, b, c, dec_w, dec_b):
    in_maps = _prep_in_maps(inp, emb, a, b, c, dec_w, dec_b)
    res = run_on_hw(in_maps).results
    logits = np.concatenate([res[i]["logits_sh"] for i in range(NCORES)], axis=1)
    logits = np.ascontiguousarray(logits.reshape(S, B, V), dtype=np.float32)
    h_last = np.ascontiguousarray(res[0]["hlastT"].T, dtype=np.float32)
    return logits, h_last


# revision 11
# speedup vs baseline: 2.0499x; 1.6416x over previous
"""CPRNN Trainium2 kernel (8-core SPMD).

Strategy:
  - The CP recurrence (sequential over S=256) is replicated on every core
    in fully-transposed orientation: h lives as hT [H, B] fp16 tiles, so
    mm1 (h@a) uses stationary `a` tiles, mm2 uses stationary `c.T` tiles,
    and tanh emits hT directly -- no per-step transposes.
  - The dominant decoder matmul [S*B, H] @ [H, V] is tensor-parallel over
    the vocab dim: every core runs the same program but receives its own
    dec_w.T shard (V/8 = 4000 columns) and writes its own logits shard.
  - bp = (emb[inp]) @ b is computed on-device per core: indirect-DMA row
    gather of fp16 emb -> PE transpose -> b-stationary matmul -> bpT fp16,
    pipelined 2 groups ahead of the recurrence.
  - All matmuls fp16 (1 cycle/row + fast weight load), fp32 PSUM
    accumulation; logits bias-add happens in fp32.

Self-contained: hardcodes all shapes; host code only reshapes/transposes.
"""
import sys
sys.path.insert(0, "/opt/trn_rl_repo")
import numpy as np

S, B, D, H, R, V = 256, 32, 1024, 1024, 256, 32000
NCORES = 8
VSH = V // NCORES          # 4000 vocab columns per core
SB = S * B                 # 8192 token rows
NG = S // 4                # 64 groups of 4 steps = 128 token rows
KD = D // 128              # 8 contraction tiles over D/H
KR = R // 128              # 2 contraction tiles over R
NT = VSH // 8              # 500 decoder free-dim columns per n-tile
LEAD = 2                   # bp chunks emitted this many groups ahead
H0VAL = 0.0                # initial hidden state fill (nonzero only for validation)
NBODY = 1                  # repeat whole body N times (timing: marginal = HW time)
SKIP_DECODER = False       # timing variant: recurrence + bp only
SKIP_REC = False           # timing variant: decoder only (hT memset)

_CACHED = {}


def _build_nc():
    import concourse.bass as bass
    import concourse.bacc as bacc
    import concourse.mybir as mybir
    import concourse.tile as tile

    fp16, fp32, i32 = mybir.dt.float16, mybir.dt.float32, mybir.dt.int32
    Tanh = mybir.ActivationFunctionType.Tanh

    nc = bacc.Bacc(None, target_bir_lowering=False)
    emb16 = nc.dram_tensor("emb16", [V, D], fp16, kind="ExternalInput")
    idx32 = nc.dram_tensor("idx32", [128, NG], i32, kind="ExternalInput")
    a16 = nc.dram_tensor("a16", [H, R], fp16, kind="ExternalInput")
    b16 = nc.dram_tensor("b16", [D, R], fp16, kind="ExternalInput")
    cT16 = nc.dram_tensor("cT16", [R, H], fp16, kind="ExternalInput")
    ident = nc.dram_tensor("ident", [128, 128], fp16, kind="ExternalInput")
    dwT = nc.dram_tensor("dwT", [H, VSH], fp16, kind="ExternalInput")
    brep = nc.dram_tensor("brep", [128, VSH], fp32, kind="ExternalInput")

    logits_sh = nc.dram_tensor("logits_sh", [SB, VSH], fp32, kind="ExternalOutput")
    hlastT = nc.dram_tensor("hlastT", [H, B], fp32, kind="ExternalOutput")

    with tile.TileContext(nc) as tc:
        with (
            tc.tile_pool(name="cst", bufs=1) as cst,
            tc.tile_pool(name="xg", bufs=3) as xgp,
            tc.tile_pool(name="xT", bufs=2) as xTp,
            tc.tile_pool(name="ghat", bufs=2) as ghp,
            tc.tile_pool(name="hT", bufs=3) as hTp,
            tc.tile_pool(name="lg", bufs=3) as lgp,
            tc.tile_pool(name="ptr", bufs=2, space="PSUM") as ptr,   # 2 banks
            tc.tile_pool(name="pbp", bufs=1, space="PSUM") as pbp,   # 1 bank
            tc.tile_pool(name="pg", bufs=1, space="PSUM") as pgp,    # 1 bank
            tc.tile_pool(name="pu", bufs=1, space="PSUM") as pup,    # 1 bank
            tc.tile_pool(name="pd", bufs=3, space="PSUM") as pdp,    # 3 banks
        ):
            # ---- constants into SBUF ----
            idx_t = cst.tile([128, NG], i32)
            nc.sync.dma_start(idx_t[:], idx32[:])
            id_t = cst.tile([128, 128], fp16)
            nc.sync.dma_start(id_t[:], ident[:])
            a_t = cst.tile([128, KD, R], fp16)
            nc.sync.dma_start(a_t[:], a16[:].rearrange("(k p) r -> p k r", p=128))
            b_t = cst.tile([128, KD, R], fp16)
            nc.sync.dma_start(b_t[:], b16[:].rearrange("(k p) r -> p k r", p=128))
            cT_t = cst.tile([128, KR, H], fp16)
            nc.sync.dma_start(cT_t[:], cT16[:].rearrange("(k p) h -> p k h", p=128))
            dw_t = cst.tile([128, KD, VSH], fp16)
            nc.sync.dma_start(dw_t[:], dwT[:].rearrange("(k p) v -> p k v", p=128))
            bias_t = cst.tile([128, VSH], fp32)
            nc.sync.dma_start(bias_t[:], brep[:])
            bpT = cst.tile([128, KR, SB], fp16)   # bp transposed: [R, tokens]
            h0_t = cst.tile([128, KD, B], fp16)
            nc.gpsimd.memset(h0_t[:], H0VAL)
            hlast_sb = cst.tile([128, KD, B], fp32)

            def bp_chunk(g):
                # gather 128 token rows of emb, transpose, project to bpT cols
                xg = xgp.tile([128, D], fp16, tag="xg")
                nc.gpsimd.indirect_dma_start(
                    out=xg[:], out_offset=None, in_=emb16[:],
                    in_offset=bass.IndirectOffsetOnAxis(ap=idx_t[:, g:g + 1], axis=0),
                )
                xT = xTp.tile([128, KD, 128], fp16, tag="xT")
                for k in range(KD):
                    tp = ptr.tile([128, 128], fp16, tag="tp")
                    nc.tensor.transpose(tp[:], xg[:, k * 128:(k + 1) * 128], id_t[:])
                    nc.any.tensor_copy(xT[:, k, :], tp[:])
                for m in range(KR):
                    acc = pbp.tile([128, 128], fp32, tag="bpacc")
                    for k in range(KD):
                        nc.tensor.matmul(acc[:], b_t[:, k, m * 128:(m + 1) * 128],
                                         xT[:, k, :], start=(k == 0), stop=(k == KD - 1))
                    nc.vector.tensor_copy(bpT[:, m, g * 128:(g + 1) * 128], acc[:])

            def decode_tiles(htile, grow, ns):
                # decoder n-tiles `ns` for token rows grow*128..+128
                for n in ns:
                    accd = pdp.tile([128, NT], fp32, tag="accd")
                    for k in range(KD):
                        nc.tensor.matmul(accd[:], htile[:, k, :],
                                         dw_t[:, k, n * NT:(n + 1) * NT],
                                         start=(k == 0), stop=(k == KD - 1))
                    lg = lgp.tile([128, NT], fp32, tag="lg")
                    nc.vector.tensor_add(lg[:], accd[:],
                                         bias_t[:, n * NT:(n + 1) * NT])
                    nc.sync.dma_start(
                        logits_sh[grow * 128:(grow + 1) * 128,
                                  n * NT:(n + 1) * NT], lg[:])

            def body():
                if not SKIP_REC:
                    for g in range(LEAD):
                        bp_chunk(g)
                hT_tiles = [None, None]  # [prev group tile, cur group tile]
                for g in range(NG):
                    if g + LEAD < NG and not SKIP_REC:
                        bp_chunk(g + LEAD)
                    hTg = hTp.tile([128, KD, 128], fp16, tag="hT")
                    hT_tiles = [hT_tiles[1], hTg]
                    if SKIP_REC:
                        nc.gpsimd.memset(hTg[:], 0.0)
                    for s in [] if SKIP_REC else range(4):
                        t = 4 * g + s
                        # previous hidden state (output of step t-1)
                        if t == 0:
                            hprev, ps = h0_t, 0
                        elif s == 0:
                            hprev, ps = hT_tiles[0], 3
                        else:
                            hprev, ps = hTg, s - 1
                        # mm1: gT[R,B] = a.T @ hT  (stationary a tiles)
                        psg = pgp.tile([128, KR * B], fp32, tag="psg")
                        for m in range(KR):
                            for k in range(KD):
                                nc.tensor.matmul(
                                    psg[:, m * B:(m + 1) * B],
                                    a_t[:, k, m * 128:(m + 1) * 128],
                                    hprev[:, k, ps * B:(ps + 1) * B],
                                    start=(k == 0), stop=(k == KD - 1))
                        # ghat = gT * bpT[:, :, t*B:(t+1)*B]  (cast to fp16)
                        gh = ghp.tile([128, KR * B], fp16, tag="gh")
                        nc.vector.tensor_mul(
                            gh[:].rearrange("p (m b) -> p m b", m=KR),
                            psg[:].rearrange("p (m b) -> p m b", m=KR),
                            bpT[:, :, t * B:(t + 1) * B])
                        # mm2: uT[H,B] = c @ ghat  (stationary cT tiles)
                        psu = pup.tile([128, KD * B], fp32, tag="psu")
                        for hh in range(KD):
                            for k in range(KR):
                                nc.tensor.matmul(
                                    psu[:, hh * B:(hh + 1) * B],
                                    cT_t[:, k, hh * 128:(hh + 1) * 128],
                                    gh[:, k * B:(k + 1) * B],
                                    start=(k == 0), stop=(k == KR - 1))
                        # tanh -> hT fp16 (decoder-ready layout)
                        for hh in range(KD):
                            nc.scalar.activation(hTg[:, hh, s * B:(s + 1) * B],
                                                 psu[:, hh * B:(hh + 1) * B], Tanh)
                        if t == S - 1:
                            for hh in range(KD):
                                nc.scalar.activation(hlast_sb[:, hh, :],
                                                     psu[:, hh * B:(hh + 1) * B], Tanh)
                        # interleave previous group's decoder tiles between
                        # steps: keeps PE dense (HAM stays warm) and fills
                        # the recurrence chain stalls
                        if g >= 1 and not SKIP_DECODER:
                            decode_tiles(hT_tiles[0], g - 1, [2 * s, 2 * s + 1])
                    if SKIP_REC and not SKIP_DECODER:
                        decode_tiles(hTg, g, range(VSH // NT))
                if not (SKIP_REC or SKIP_DECODER):
                    decode_tiles(hT_tiles[1], NG - 1, range(VSH // NT))
                for hh in range(KD):
                    nc.sync.dma_start(hlastT[hh * 128:(hh + 1) * 128, :],
                                      hlast_sb[:, hh, :])

            for _rep in range(NBODY):
                body()
    nc.finalize()
    return nc


def _get_nc():
    if "nc" not in _CACHED:
        _CACHED["nc"] = _build_nc()
    return _CACHED["nc"]


def _prep_in_maps(inp, emb, a, b, c, dec_w, dec_b):
    f16 = np.float16
    emb16 = np.ascontiguousarray(emb, dtype=f16)
    a16 = np.ascontiguousarray(a, dtype=f16)
    b16 = np.ascontiguousarray(b, dtype=f16)
    cT16 = np.ascontiguousarray(np.asarray(c, dtype=np.float32).T, dtype=f16)
    ident = np.eye(128, dtype=f16)
    # token (g*128 + p) at idx32[p, g]
    idx32 = np.ascontiguousarray(
        np.asarray(inp, dtype=np.int64).reshape(SB).reshape(NG, 128).T
    ).astype(np.int32)
    dwT = np.asarray(dec_w, dtype=np.float32).T  # [H, V]
    dec_b = np.asarray(dec_b, dtype=np.float32)
    common = dict(emb16=emb16, idx32=idx32, a16=a16, b16=b16, cT16=cT16, ident=ident)
    in_maps = []
    for core in range(NCORES):
        sl = slice(core * VSH, (core + 1) * VSH)
        m = dict(common)
        m["dwT"] = np.ascontiguousarray(dwT[:, sl], dtype=f16)
        m["brep"] = np.ascontiguousarray(
            np.broadcast_to(dec_b[sl][None, :], (128, VSH)), dtype=np.float32)
        in_maps.append(m)
    return in_maps


def run_on_hw(in_maps, **kwargs):
    from concourse.bass_utils import run_bass_kernel_spmd
    nc = _get_nc()
    return run_bass_kernel_spmd(nc, in_maps, list(range(NCORES)), **kwargs)


def kernel(inp, emb, a, b, c, dec_w, dec_b):
    in_maps = _prep_in_maps(inp, emb, a, b, c, dec_w, dec_b)
    res = run_on_hw(in_maps).results
    logits = np.concatenate([res[i]["logits_sh"] for i in range(NCORES)], axis=1)
    logits = np.ascontiguousarray(logits.reshape(S, B, V), dtype=np.float32)
    h_last = np.ascontiguousarray(res[0]["hlastT"].T, dtype=np.float32)
    return logits, h_last


# revision 14
# speedup vs baseline: 2.0819x; 1.0156x over previous
"""CPRNN Trainium2 kernel (8-core SPMD).

Strategy:
  - The CP recurrence (sequential over S=256) is replicated on every core
    in fully-transposed orientation: h lives as hT [H, B] fp16 tiles, so
    mm1 (h@a) uses stationary `a` tiles, mm2 uses stationary `c.T` tiles,
    and tanh emits hT directly -- no per-step transposes.
  - The dominant decoder matmul [S*B, H] @ [H, V] is tensor-parallel over
    the vocab dim: every core runs the same program but receives its own
    dec_w.T shard (V/8 = 4000 columns) and writes its own logits shard.
  - bp = (emb[inp]) @ b is computed on-device per core: indirect-DMA row
    gather of fp16 emb -> PE transpose -> b-stationary matmul -> bpT fp16,
    pipelined 2 groups ahead of the recurrence.
  - All matmuls fp16 (1 cycle/row + fast weight load), fp32 PSUM
    accumulation; logits bias-add happens in fp32.

Self-contained: hardcodes all shapes; host code only reshapes/transposes.
"""
import sys
sys.path.insert(0, "/opt/trn_rl_repo")
import numpy as np

S, B, D, H, R, V = 256, 32, 1024, 1024, 256, 32000
NCORES = 8
VSH = V // NCORES          # 4000 vocab columns per core
SB = S * B                 # 8192 token rows
NG = S // 4                # 64 groups of 4 steps = 128 token rows
KD = D // 128              # 8 contraction tiles over D/H
KR = R // 128              # 2 contraction tiles over R
NT = VSH // 8              # 500 decoder free-dim columns per n-tile
LEAD = 2                   # bp chunks emitted this many groups ahead
H0VAL = 0.0                # initial hidden state fill (nonzero only for validation)
NBODY = 1                  # repeat whole body N times (timing: marginal = HW time)
SKIP_DECODER = False       # timing variant: recurrence + bp only
SKIP_REC = False           # timing variant: decoder only (hT memset)

_CACHED = {}


def _build_nc():
    import concourse.bass as bass
    import concourse.bacc as bacc
    import concourse.mybir as mybir
    import concourse.tile as tile

    fp16, fp32, i32 = mybir.dt.float16, mybir.dt.float32, mybir.dt.int32
    Tanh = mybir.ActivationFunctionType.Tanh

    nc = bacc.Bacc(None, target_bir_lowering=False)
    emb16 = nc.dram_tensor("emb16", [V, D], fp16, kind="ExternalInput")
    idx32 = nc.dram_tensor("idx32", [128, NG], i32, kind="ExternalInput")
    a16 = nc.dram_tensor("a16", [H, R], fp16, kind="ExternalInput")
    b16 = nc.dram_tensor("b16", [D, R], fp16, kind="ExternalInput")
    cT16 = nc.dram_tensor("cT16", [R, H], fp16, kind="ExternalInput")
    ident = nc.dram_tensor("ident", [128, 128], fp16, kind="ExternalInput")
    dwT = nc.dram_tensor("dwT", [H, VSH], fp16, kind="ExternalInput")
    brep = nc.dram_tensor("brep", [128, VSH], fp32, kind="ExternalInput")

    logits_sh = nc.dram_tensor("logits_sh", [SB, VSH], fp32, kind="ExternalOutput")
    hlastT = nc.dram_tensor("hlastT", [H, B], fp32, kind="ExternalOutput")

    with tile.TileContext(nc) as tc:
        with (
            tc.tile_pool(name="cst", bufs=1) as cst,
            tc.tile_pool(name="xg", bufs=3) as xgp,
            tc.tile_pool(name="xT", bufs=2) as xTp,
            tc.tile_pool(name="ghat", bufs=2) as ghp,
            tc.tile_pool(name="hT", bufs=3) as hTp,
            tc.tile_pool(name="lg", bufs=3) as lgp,
            tc.tile_pool(name="ptr", bufs=2, space="PSUM") as ptr,   # 2 banks
            tc.tile_pool(name="pbp", bufs=1, space="PSUM") as pbp,   # 1 bank
            tc.tile_pool(name="pg", bufs=1, space="PSUM") as pgp,    # 1 bank
            tc.tile_pool(name="pu", bufs=1, space="PSUM") as pup,    # 1 bank
            tc.tile_pool(name="pd", bufs=3, space="PSUM") as pdp,    # 3 banks
        ):
            # ---- constants into SBUF ----
            idx_t = cst.tile([128, NG], i32)
            nc.sync.dma_start(idx_t[:], idx32[:])
            id_t = cst.tile([128, 128], fp16)
            nc.sync.dma_start(id_t[:], ident[:])
            a_t = cst.tile([128, KD, R], fp16)
            nc.sync.dma_start(a_t[:], a16[:].rearrange("(k p) r -> p k r", p=128))
            b_t = cst.tile([128, KD, R], fp16)
            nc.sync.dma_start(b_t[:], b16[:].rearrange("(k p) r -> p k r", p=128))
            cT_t = cst.tile([128, KR, H], fp16)
            nc.sync.dma_start(cT_t[:], cT16[:].rearrange("(k p) h -> p k h", p=128))
            dw_t = cst.tile([128, KD, VSH], fp16)
            nc.sync.dma_start(dw_t[:], dwT[:].rearrange("(k p) v -> p k v", p=128))
            bias_t = cst.tile([128, VSH], fp32)
            nc.sync.dma_start(bias_t[:], brep[:])
            bpT = cst.tile([128, KR, SB], fp16)   # bp transposed: [R, tokens]
            h0_t = cst.tile([128, KD, B], fp16)
            nc.gpsimd.memset(h0_t[:], H0VAL)
            hlast_sb = cst.tile([128, KD, B], fp32)

            def bp_chunk(c):
                # gather 256 token rows of emb (2 indirect DMAs), transpose,
                # project to bpT cols c*256..(c+1)*256
                xg = xgp.tile([128, 2, D], fp16, tag="xg")
                for j in range(2):
                    nc.gpsimd.indirect_dma_start(
                        out=xg[:, j, :], out_offset=None, in_=emb16[:],
                        in_offset=bass.IndirectOffsetOnAxis(
                            ap=idx_t[:, 2 * c + j:2 * c + j + 1], axis=0),
                    )
                xT = xTp.tile([128, KD, 256], fp16, tag="xT")
                for k in range(KD):
                    for j in range(2):
                        tp = ptr.tile([128, 128], fp16, tag="tp")
                        nc.tensor.transpose(
                            tp[:], xg[:, j, k * 128:(k + 1) * 128], id_t[:])
                        nc.any.tensor_copy(xT[:, k, j * 128:(j + 1) * 128], tp[:])
                for m in range(KR):
                    acc = pbp.tile([128, 256], fp32, tag="bpacc")
                    for k in range(KD):
                        nc.tensor.matmul(acc[:], b_t[:, k, m * 128:(m + 1) * 128],
                                         xT[:, k, :], start=(k == 0), stop=(k == KD - 1))
                    nc.vector.tensor_copy(bpT[:, m, c * 256:(c + 1) * 256], acc[:])

            def decode_tiles(htile, grow, ns):
                # decoder n-tiles `ns` for token rows grow*128..+128
                for n in ns:
                    accd = pdp.tile([128, NT], fp32, tag="accd")
                    for k in range(KD):
                        nc.tensor.matmul(accd[:], htile[:, k, :],
                                         dw_t[:, k, n * NT:(n + 1) * NT],
                                         start=(k == 0), stop=(k == KD - 1))
                    lg = lgp.tile([128, NT], fp32, tag="lg")
                    nc.vector.tensor_add(lg[:], accd[:],
                                         bias_t[:, n * NT:(n + 1) * NT])
                    nc.sync.dma_start(
                        logits_sh[grow * 128:(grow + 1) * 128,
                                  n * NT:(n + 1) * NT], lg[:])

            def body():
                NC2 = NG // 2  # 256-token bp chunks
                if not SKIP_REC:
                    for c in range(LEAD):
                        bp_chunk(c)
                hT_tiles = [None, None]  # [prev group tile, cur group tile]
                for g in range(NG):
                    if g % 2 == 0 and g // 2 + LEAD < NC2 and not SKIP_REC:
                        bp_chunk(g // 2 + LEAD)
                    hTg = hTp.tile([128, KD, 128], fp16, tag="hT")
                    hT_tiles = [hT_tiles[1], hTg]
                    if SKIP_REC:
                        nc.gpsimd.memset(hTg[:], 0.0)
                    for s in [] if SKIP_REC else range(4):
                        t = 4 * g + s
                        # previous hidden state (output of step t-1)
                        if t == 0:
                            hprev, ps = h0_t, 0
                        elif s == 0:
                            hprev, ps = hT_tiles[0], 3
                        else:
                            hprev, ps = hTg, s - 1
                        # mm1: gT[R,B] = a.T @ hT  (stationary a tiles)
                        psg = pgp.tile([128, KR * B], fp32, tag="psg")
                        for m in range(KR):
                            for k in range(KD):
                                nc.tensor.matmul(
                                    psg[:, m * B:(m + 1) * B],
                                    a_t[:, k, m * 128:(m + 1) * 128],
                                    hprev[:, k, ps * B:(ps + 1) * B],
                                    start=(k == 0), stop=(k == KD - 1))
                        # ghat = gT * bpT[:, :, t*B:(t+1)*B]  (cast to fp16)
                        gh = ghp.tile([128, KR * B], fp16, tag="gh")
                        nc.vector.tensor_mul(
                            gh[:].rearrange("p (m b) -> p m b", m=KR),
                            psg[:].rearrange("p (m b) -> p m b", m=KR),
                            bpT[:, :, t * B:(t + 1) * B])
                        # mm2: uT[H,B] = c @ ghat  (stationary cT tiles)
                        psu = pup.tile([128, KD * B], fp32, tag="psu")
                        for hh in range(KD):
                            for k in range(KR):
                                nc.tensor.matmul(
                                    psu[:, hh * B:(hh + 1) * B],
                                    cT_t[:, k, hh * 128:(hh + 1) * 128],
                                    gh[:, k * B:(k + 1) * B],
                                    start=(k == 0), stop=(k == KR - 1))
                        # tanh -> hT fp16 (decoder-ready layout), one ACT op
                        nc.scalar.activation(
                            hTg[:, :, s * B:(s + 1) * B],
                            psu[:].rearrange("p (k b) -> p k b", k=KD), Tanh)
                        if t == S - 1:
                            for hh in range(KD):
                                nc.scalar.activation(hlast_sb[:, hh, :],
                                                     psu[:, hh * B:(hh + 1) * B], Tanh)
                        # interleave previous group's decoder tiles between
                        # steps: keeps PE dense (HAM stays warm) and fills
                        # the recurrence chain stalls
                        if g >= 1 and not SKIP_DECODER:
                            decode_tiles(hT_tiles[0], g - 1, [2 * s, 2 * s + 1])
                    if SKIP_REC and not SKIP_DECODER:
                        decode_tiles(hTg, g, range(VSH // NT))
                if not (SKIP_REC or SKIP_DECODER):
                    decode_tiles(hT_tiles[1], NG - 1, range(VSH // NT))
                for hh in range(KD):
                    nc.sync.dma_start(hlastT[hh * 128:(hh + 1) * 128, :],
                                      hlast_sb[:, hh, :])

            for _rep in range(NBODY):
                body()
    nc.finalize()
    return nc


def _get_nc():
    if "nc" not in _CACHED:
        _CACHED["nc"] = _build_nc()
    return _CACHED["nc"]


def _prep_in_maps(inp, emb, a, b, c, dec_w, dec_b):
    f16 = np.float16
    emb16 = np.ascontiguousarray(emb, dtype=f16)
    a16 = np.ascontiguousarray(a, dtype=f16)
    b16 = np.ascontiguousarray(b, dtype=f16)
    cT16 = np.ascontiguousarray(np.asarray(c, dtype=np.float32).T, dtype=f16)
    ident = np.eye(128, dtype=f16)
    # token (g*128 + p) at idx32[p, g]
    idx32 = np.ascontiguousarray(
        np.asarray(inp, dtype=np.int64).reshape(SB).reshape(NG, 128).T
    ).astype(np.int32)
    dwT = np.asarray(dec_w, dtype=np.float32).T  # [H, V]
    dec_b = np.asarray(dec_b, dtype=np.float32)
    common = dict(emb16=emb16, idx32=idx32, a16=a16, b16=b16, cT16=cT16, ident=ident)
    in_maps = []
    for core in range(NCORES):
        sl = slice(core * VSH, (core + 1) * VSH)
        m = dict(common)
        m["dwT"] = np.ascontiguousarray(dwT[:, sl], dtype=f16)
        m["brep"] = np.ascontiguousarray(
            np.broadcast_to(dec_b[sl][None, :], (128, VSH)), dtype=np.float32)
        in_maps.append(m)
    return in_maps


def run_on_hw(in_maps, **kwargs):
    from concourse.bass_utils import run_bass_kernel_spmd
    nc = _get_nc()
    return run_bass_kernel_spmd(nc, in_maps, list(range(NCORES)), **kwargs)


def kernel(inp, emb, a, b, c, dec_w, dec_b):
    in_maps = _prep_in_maps(inp, emb, a, b, c, dec_w, dec_b)
    res = run_on_hw(in_maps).results
    logits = np.concatenate([res[i]["logits_sh"] for i in range(NCORES)], axis=1)
    logits = np.ascontiguousarray(logits.reshape(S, B, V), dtype=np.float32)
    h_last = np.ascontiguousarray(res[0]["hlastT"].T, dtype=np.float32)
    return logits, h_last
